# revision 27
# baseline (speedup 1.0000x reference)
"""MixFFN MoE-routing kernel for Trainium2 (8 NeuronCores, token-parallel).

Math (per token block):
    logits = x @ gate_w.T ; probs = softmax(logits); top2 -> ew [N, E] (dense, rows sum to 1)
    CW1 = x @ W1.T ; CW3 = x @ W3.T
    per expert e:
        w1_e = CW1 + (x @ A1e.T) @ B1e.T
        w3_e = CW3 + (x @ A3e.T) @ B3e.T
        h_e  = silu(w1_e) * w3_e
    out = (sum_e ew_e * h_e) @ W2.T + sum_e ((ew_e * h_e) @ A2e.T) @ B2e.T

Key restructuring vs the reference scan: row-scaling by ew commutes with the
right matmuls, so the big W2 GEMM runs once on H = sum_e ew_e*h_e instead of
once per expert.  The u-projection (A2 contraction) is computed from the
unscaled h_e and column-scaled by ew afterwards (column scaling commutes with
contraction over dff).

Sharding: token-parallel.  Each of the 8 cores gets N/8 = 512 tokens and a
replicated copy of all weights; outputs are disjoint row blocks (no
collectives).  All layout transposes / dtype casts are done host-side.

On-chip layout: feature-on-partition ("transposed"), activations [feat, tok].
"""

import numpy as np

# problem dims (hardcoded per harness contract)
N, D, DFF, E, KTOP, R = 4096, 2048, 8192, 8, 2, 16
NCORES = 8
P = 128

_CACHE = {}


def build_bass(D_=D, DFF_=DFF, E_=E, R_=R, NTOK=N // NCORES):
    """Build the per-core Bass program (same SPMD program on every core)."""
    import concourse.bass as bass
    import concourse.mybir as mybir
    from concourse import bacc
    from concourse.tile import TileContext
    from concourse.masks import make_identity

    dt = mybir.dt
    op = mybir.AluOpType
    AF = mybir.ActivationFunctionType

    KD = D_ // P      # contraction tiles over D
    KF = DFF_ // P    # dff tiles
    MD = D_ // P      # output d tiles
    TT = NTOK // P    # token tiles
    ER = E_ * R_      # stacked expert-rank dim (=128 at full size)

    nc = bacc.Bacc("TRN2", target_bir_lowering=False, debug=False)

    # ---- DRAM I/O ----
    x_bf = nc.dram_tensor("x_bf", [D_, NTOK], dt.bfloat16, kind="ExternalInput")
    x_f = nc.dram_tensor("x_f", [D_, NTOK], dt.float32, kind="ExternalInput")
    gate = nc.dram_tensor("gate", [D_, E_], dt.float32, kind="ExternalInput")
    w1t = nc.dram_tensor("w1t", [D_, DFF_], dt.bfloat16, kind="ExternalInput")
    w3t = nc.dram_tensor("w3t", [D_, DFF_], dt.bfloat16, kind="ExternalInput")
    w2t = nc.dram_tensor("w2t", [DFF_, D_], dt.bfloat16, kind="ExternalInput")
    # A1/A3 packed even/odd with 32-aligned expert slots:
    # a1p[:, par, 32*j : 32*j+16] = A1[2*j+par].T  (zeros elsewhere)
    a1p = nc.dram_tensor("a1p", [D_, 2, P], dt.bfloat16, kind="ExternalInput")
    a3p = nc.dram_tensor("a3p", [D_, 2, P], dt.bfloat16, kind="ExternalInput")
    b1s = nc.dram_tensor("b1s", [R_, E_, DFF_], dt.bfloat16, kind="ExternalInput")
    b3s = nc.dram_tensor("b3s", [R_, E_, DFF_], dt.bfloat16, kind="ExternalInput")
    a2s = nc.dram_tensor("a2s", [DFF_, ER], dt.bfloat16, kind="ExternalInput")
    b2s = nc.dram_tensor("b2s", [R_, E_, D_], dt.bfloat16, kind="ExternalInput")
    out_t = nc.dram_tensor("out_t", [D_, NTOK], dt.float32, kind="ExternalOutput")

    with TileContext(nc) as tc:
        with (
            tc.tile_pool(name="persist", bufs=1) as persist,
            tc.tile_pool(name="psum_cw", bufs=1, space="PSUM") as ppool_cw,
            tc.tile_pool(name="psum_d", bufs=2, space="PSUM") as ppool_d,
            tc.tile_pool(name="psum_u", bufs=1, space="PSUM") as ppool_u,
            tc.tile_pool(name="dram", bufs=1, space="DRAM") as dpool,
        ):
            # ---------- persistent tiles (created upfront so the persist
            # pool's footprint is settled before scoped pools stack above) ----
            xbf = persist.tile([P, KD, NTOK], dt.bfloat16)
            nc.sync.dma_start(out=xbf, in_=x_bf.rearrange("(k p) t -> p k t", p=P))
            ident = persist.tile([P, P], dt.bfloat16)
            make_identity(nc, ident)
            ident_f = persist.tile([P, P], dt.float32)
            make_identity(nc, ident_f)
            # H accumulator (bf16) for the whole dff range
            h_big = persist.tile([P, KF, NTOK], dt.bfloat16)
            ewT_sb = persist.tile([E_, NTOK], dt.bfloat16)
            ew_b = []
            for e in range(E_):
                ewb_t = persist.tile([P, NTOK], dt.bfloat16, tag=f"ewb{e}")
                ew_b.append(ewb_t)
            t1sb, t3sb = [None] * E_, [None] * E_
            for e in range(E_):
                t1_t = persist.tile([R_, NTOK], dt.bfloat16, tag=f"t1_{e}")
                t1sb[e] = t1_t
                t3_t = persist.tile([R_, NTOK], dt.bfloat16, tag=f"t3_{e}")
                t3sb[e] = t3_t
            uw = []
            for e in range(E_):
                uw_t = persist.tile([R_, NTOK], dt.bfloat16, tag=f"uw{e}")
                uw.append(uw_t)

            # ---------- phase 0: routing + lora-down projections ----------
            ew_td = dpool.tile([E_, NTOK], dt.bfloat16)
            p0_cm = tc.tile_pool(name="p0", bufs=3)
            p0 = p0_cm.__enter__()
            xf = p0.tile([P, KD, NTOK], dt.float32, bufs=1)
            nc.sync.dma_start(out=xf, in_=x_f.rearrange("(k p) t -> p k t", p=P))
            gsb = p0.tile([P, KD, E_], dt.float32, bufs=1)
            nc.sync.dma_start(out=gsb, in_=gate.rearrange("(k p) e -> p k e", p=P))
            a1sb = p0.tile([P, KD, 2, P], dt.bfloat16, bufs=1)
            nc.sync.dma_start(
                out=a1sb, in_=a1p.rearrange("(k p) g c -> p k g c", p=P)
            )
            a3sb = p0.tile([P, KD, 2, P], dt.bfloat16, bufs=1)
            nc.sync.dma_start(
                out=a3sb, in_=a3p.rearrange("(k p) g c -> p k g c", p=P)
            )
            ew_pool = p0
            for tt in range(TT):
                lg = ppool_d.tile([P, E_], dt.float32, tag="d1")
                for k in range(KD):
                    nc.tensor.matmul(
                        lg,
                        lhsT=xf[:, k, tt * P:(tt + 1) * P],
                        rhs=gsb[:, k, :],
                        start=(k == 0),
                        stop=(k == KD - 1),
                    )
                l_sb = ew_pool.tile([P, E_], dt.float32, tag="lsb")
                nc.vector.tensor_copy(l_sb, lg)
                m1 = ew_pool.tile([P, 1], dt.float32, tag="m1")
                nc.vector.reduce_max(m1, l_sb, axis=mybir.AxisListType.X)
                nm1 = ew_pool.tile([P, 1], dt.float32, tag="nm1")
                nc.vector.tensor_scalar_mul(nm1, m1, -1.0)
                mask1 = ew_pool.tile([P, E_], dt.float32, tag="mask1")
                nc.vector.tensor_scalar(
                    mask1, l_sb, scalar1=m1, scalar2=None, op0=op.is_equal
                )
                l2 = ew_pool.tile([P, E_], dt.float32, tag="l2")
                # l2 = mask1 * (-1e30) + l
                nc.vector.scalar_tensor_tensor(
                    l2, in0=mask1, scalar=-1e30, in1=l_sb, op0=op.mult, op1=op.add
                )
                m2 = ew_pool.tile([P, 1], dt.float32, tag="m2")
                nc.vector.reduce_max(m2, l2, axis=mybir.AxisListType.X)
                mask2 = ew_pool.tile([P, E_], dt.float32, tag="mask2")
                nc.vector.tensor_scalar(
                    mask2, l2, scalar1=m2, scalar2=None, op0=op.is_equal
                )
                mask = ew_pool.tile([P, E_], dt.float32, tag="mask")
                nc.vector.tensor_tensor(mask, mask1, mask2, op=op.add)
                pexp = ew_pool.tile([P, E_], dt.float32, tag="pexp")
                nc.scalar.activation(pexp, l_sb, AF.Exp, bias=nm1, scale=1.0)
                pm = ew_pool.tile([P, E_], dt.float32, tag="pm")
                nc.vector.tensor_tensor(pm, pexp, mask, op=op.mult)
                den = ew_pool.tile([P, 1], dt.float32, tag="den")
                nc.vector.reduce_sum(den, pm, axis=mybir.AxisListType.X)
                rec = ew_pool.tile([P, 1], dt.float32, tag="rec")
                nc.vector.reciprocal(rec, den)
                ewt = ew_pool.tile([P, E_], dt.float32, tag="ewt")
                nc.vector.tensor_scalar_mul(ewt, pm, rec)
                # transpose [P, E] -> [E, P] and collect into ewT
                ewtp = ppool_d.tile([E_, P], dt.float32, tag="d3", bufs=1)
                nc.tensor.transpose(ewtp, ewt, ident_f)
                nc.scalar.copy(ewT_sb[:, tt * P:(tt + 1) * P], ewtp)

            nc.sync.dma_start(out=ew_td, in_=ewT_sb)
            # broadcast ew rows across partitions: EW_b[e] [P, NTOK]
            for e in range(E_):
                src = bass.AP(
                    tensor=ew_td.tensor,
                    offset=ew_td.offset + e * NTOK,
                    ap=[[0, P], [1, NTOK]],
                )
                nc.gpsimd.dma_start(out=ew_b[e], in_=src)

            # ---------- T1/T3 = stacked per-expert lora-down projections ----------
            # expert 2*j+par sits at rows 32*j..32*j+16 of the `par` chain
            for asb, tlist in ((a1sb, t1sb), (a3sb, t3sb)):
                for par in range(2):
                    tp = ppool_d.tile([P, NTOK], dt.float32, tag="d1")
                    for k in range(KD):
                        nc.tensor.matmul(
                            tp,
                            lhsT=asb[:, k, par, :],
                            rhs=xbf[:, k, :],
                            start=(k == 0),
                            stop=(k == KD - 1),
                        )
                    for j in range(E_ // 2):
                        e = 2 * j + par
                        nc.scalar.copy(tlist[e], tp[32 * j:32 * j + R_, :])

            p0_cm.__exit__(None, None, None)
            stream_cm = tc.tile_pool(name="stream", bufs=2)
            stream = stream_cm.__enter__()

            # ---------- U accumulators (per-expert lora-up of h, unscaled) ----------
            u_ps_a = ppool_u.tile([P, NTOK], dt.float32, tag="ua")
            u_ps_b = ppool_u.tile([P, NTOK], dt.float32, tag="ub")
            u_ps = [u_ps_a, u_ps_b]

            # ---------- main dff loop ----------
            ttc = 0  # round-robin counter for p/q engine assignment
            for kt in range(KF):
                # CW1/CW3 for this dff tile
                w1sl = stream.tile([P, KD, P], dt.bfloat16, tag="w1sl")
                nc.sync.dma_start(
                    out=w1sl,
                    in_=w1t.rearrange("(k p) f -> p k f", p=P)[
                        :, :, kt * P:(kt + 1) * P
                    ],
                )
                w3sl = stream.tile([P, KD, P], dt.bfloat16, tag="w3sl")
                nc.sync.dma_start(
                    out=w3sl,
                    in_=w3t.rearrange("(k p) f -> p k f", p=P)[
                        :, :, kt * P:(kt + 1) * P
                    ],
                )
                cw1p = ppool_cw.tile([P, NTOK], dt.float32, tag="cw1")
                cw3p = ppool_cw.tile([P, NTOK], dt.float32, tag="cw3")
                for k in range(KD):
                    nc.tensor.matmul(
                        cw1p, lhsT=w1sl[:, k, :], rhs=xbf[:, k, :],
                        start=(k == 0), stop=(k == KD - 1),
                    )
                for k in range(KD):
                    nc.tensor.matmul(
                        cw3p, lhsT=w3sl[:, k, :], rhs=xbf[:, k, :],
                        start=(k == 0), stop=(k == KD - 1),
                    )
                cw1 = stream.tile([P, NTOK], dt.bfloat16, tag="cw1s")
                nc.scalar.copy(cw1, cw1p)
                cw3 = stream.tile([P, NTOK], dt.bfloat16, tag="cw3s")
                nc.scalar.copy(cw3, cw3p)

                # per-kt lora weights
                b1kt = stream.tile([R_, E_, P], dt.bfloat16, tag="b1kt")
                nc.sync.dma_start(out=b1kt, in_=b1s[:, :, kt * P:(kt + 1) * P])
                b3kt = stream.tile([R_, E_, P], dt.bfloat16, tag="b3kt")
                nc.sync.dma_start(out=b3kt, in_=b3s[:, :, kt * P:(kt + 1) * P])
                a2kt = stream.tile([P, ER], dt.bfloat16, tag="a2kt")
                nc.sync.dma_start(out=a2kt, in_=a2s[kt * P:(kt + 1) * P, :])

                hp = ppool_cw.tile([P, NTOK], dt.float32, tag="hp")

                for e in range(E_):
                    d1p = ppool_d.tile([P, NTOK], dt.float32, tag="d1")
                    nc.tensor.matmul(
                        d1p, lhsT=b1kt[:, e, :], rhs=t1sb[e], start=True, stop=True
                    )
                    d3p = ppool_d.tile([P, NTOK], dt.float32, tag="d3", bufs=1)
                    nc.tensor.matmul(
                        d3p, lhsT=b3kt[:, e, :], rhs=t3sb[e], start=True, stop=True
                    )
                    w1e = stream.tile([P, NTOK], dt.bfloat16, tag="w1e", bufs=3)
                    nc.vector.tensor_tensor(w1e, cw1, d1p, op=op.add)
                    s_e = stream.tile([P, NTOK], dt.bfloat16, tag="s_e", bufs=3)
                    nc.scalar.activation(s_e, w1e, AF.Silu)
                    w3e = stream.tile([P, NTOK], dt.bfloat16, tag="w3e", bufs=3)
                    nc.vector.tensor_tensor(w3e, cw3, d3p, op=op.add)
                    p_e = stream.tile([P, NTOK], dt.bfloat16, tag="p_e", bufs=3)
                    eng = nc.vector if ttc % 4 == 0 else nc.gpsimd
                    ttc += 1
                    eng.tensor_tensor(p_e, s_e, w3e, op=op.mult)
                    q_e = stream.tile([P, NTOK], dt.bfloat16, tag="q_e", bufs=3)
                    eng = nc.vector if ttc % 4 == 0 else nc.gpsimd
                    ttc += 1
                    eng.tensor_tensor(q_e, p_e, ew_b[e], op=op.mult)
                    # H += q_e (PE identity-matmul accumulate, fp32 in psum)
                    nc.tensor.matmul(
                        hp, lhsT=ident, rhs=q_e, start=(e == 0), stop=(e == E_ - 1)
                    )
                    # U[e] += A2e.T-contraction of (unscaled) p_e
                    j = e // 2
                    nc.tensor.matmul(
                        u_ps[e % 2][32 * j:32 * j + R_, :],
                        lhsT=a2kt[:, e * R_:(e + 1) * R_],
                        rhs=p_e,
                        start=(kt == 0),
                        stop=(kt == KF - 1),
                        tile_position=(0, 32 * j),
                    )
                nc.scalar.copy(h_big[:, kt, :], hp)

            # ---------- Uw: apply ew column scaling to U ----------
            for e in range(E_):
                j = e // 2
                nc.vector.tensor_tensor(
                    uw[e], u_ps[e % 2][32 * j:32 * j + R_, :], ew_b[e][0:R_, :],
                    op=op.mult,
                )

            # ---------- output GEMM: out = W2 @ H + sum_e B2e @ Uw_e ----------
            KH = KF // 2
            w2r = w2t.rearrange("(k p) n -> p k n", p=P)
            for m in range(MD):
                outp = ppool_d.tile([P, NTOK], dt.float32, tag="d1")
                for h in range(2):
                    w2m = stream.tile([P, KH, P], dt.bfloat16, tag="w2m")
                    nc.sync.dma_start(
                        out=w2m,
                        in_=w2r[:, h * KH:(h + 1) * KH, m * P:(m + 1) * P],
                    )
                    for kk in range(KH):
                        kt = h * KH + kk
                        nc.tensor.matmul(
                            outp, lhsT=w2m[:, kk, :], rhs=h_big[:, kt, :],
                            start=(kt == 0), stop=False,
                        )
                b2m = stream.tile([R_, E_, P], dt.bfloat16, tag="b2m")
                nc.sync.dma_start(out=b2m, in_=b2s[:, :, m * P:(m + 1) * P])
                for e in range(E_):
                    nc.tensor.matmul(
                        outp, lhsT=b2m[:, e, :], rhs=uw[e],
                        start=False, stop=(e == E_ - 1),
                    )
                osb = stream.tile([P, NTOK], dt.float32, tag="osb")
                nc.scalar.copy(osb, outp)
                nc.sync.dma_start(out=out_t[m * P:(m + 1) * P, :], in_=osb)

            stream_cm.__exit__(None, None, None)

    nc.compile()
    return nc


def _pack_a_evenodd(A):
    """A [E, R, D] -> [D, 2, 128] with A[2j+par].T at [:, par, 32j:32j+16]."""
    E_, R_, D_ = A.shape
    out = np.zeros((D_, 2, 128), A.dtype)
    for e in range(E_):
        par, j = e % 2, e // 2
        out[:, par, 32 * j:32 * j + R_] = A[e].T
    return np.ascontiguousarray(out)


def _prep_inputs(x, W1, W3, W2, gate_w, A1, B1, A3, B3, A2, B2):
    """Host-side packing: transposes + casts, shared across cores."""
    import ml_dtypes

    bf16 = ml_dtypes.bfloat16
    f32 = np.float32

    xT = np.ascontiguousarray(np.asarray(x, f32).T)            # [D, N]
    shared = {
        "gate": np.ascontiguousarray(np.asarray(gate_w, f32).T),   # [D, E]
        "w1t": np.ascontiguousarray(np.asarray(W1, f32).T.astype(bf16)),
        "w3t": np.ascontiguousarray(np.asarray(W3, f32).T.astype(bf16)),
        "w2t": np.ascontiguousarray(np.asarray(W2, f32).T.astype(bf16)),
        "a1p": _pack_a_evenodd(np.asarray(A1, f32)).astype(bf16),
        "a3p": _pack_a_evenodd(np.asarray(A3, f32)).astype(bf16),
        "b1s": np.ascontiguousarray(np.asarray(B1, f32).transpose(2, 0, 1)).astype(bf16),
        "b3s": np.ascontiguousarray(np.asarray(B3, f32).transpose(2, 0, 1)).astype(bf16),
        "a2s": np.ascontiguousarray(
            np.asarray(A2, f32).transpose(2, 0, 1).reshape(A2.shape[2], -1)
        ).astype(bf16),
        "b2s": np.ascontiguousarray(np.asarray(B2, f32).transpose(2, 0, 1)).astype(bf16),
    }
    ntok = xT.shape[1] // NCORES
    in_maps = []
    for c in range(NCORES):
        sl = np.ascontiguousarray(xT[:, c * ntok:(c + 1) * ntok])
        m = dict(shared)
        m["x_f"] = sl
        m["x_bf"] = sl.astype(bf16)
        in_maps.append(m)
    return in_maps


def _ensure_compiled():
    """Build the Bass program and a jitted 8-core shard_map executor.

    Mirrors concourse.bass2jax.run_bass_via_pjrt, but caches the jitted
    callable and keeps real inputs un-donated so device buffers can be
    reused across calls (for timing)."""
    if "exec" in _CACHE:
        return _CACHE["exec"]

    import jax
    import concourse.mybir as mybir
    from concourse import bass2jax
    from jax.experimental.shard_map import shard_map
    from jax.sharding import Mesh, PartitionSpec

    nc = build_bass()
    bass2jax.install_neuronx_cc_hook()

    partition_name = (
        nc.partition_id_tensor.name if nc.partition_id_tensor else None
    )
    in_names, out_names, out_avals, zero_outs = [], [], [], []
    for alloc in nc.m.functions[0].allocations:
        if not isinstance(alloc, mybir.MemoryLocationSet):
            continue
        name = alloc.memorylocations[0].name
        if alloc.kind == "ExternalInput":
            if name != partition_name:
                in_names.append(name)
        elif alloc.kind == "ExternalOutput":
            np_dtype = mybir.dt.np(alloc.dtype)
            out_names.append(name)
            out_avals.append(
                jax.core.ShapedArray(tuple(alloc.tensor_shape), np_dtype)
            )
            zero_outs.append(np.zeros(tuple(alloc.tensor_shape), np_dtype))

    n_params = len(in_names)
    n_outs = len(out_names)
    all_names = in_names + out_names
    if partition_name is not None:
        all_names = all_names + [partition_name]

    def _body(*args):
        operands = list(args)
        if partition_name is not None:
            operands.append(bass2jax.partition_id_tensor())
        outs = bass2jax._bass_exec_p.bind(
            *operands,
            out_avals=tuple(out_avals),
            in_names=tuple(all_names),
            out_names=tuple(out_names),
            lowering_input_output_aliases=(),
            sim_require_finite=True,
            sim_require_nnan=True,
            nc=nc,
        )
        return tuple(outs)

    devices = jax.devices()[:NCORES]
    mesh = Mesh(np.asarray(devices), ("core",))
    in_specs = (PartitionSpec("core"),) * (n_params + n_outs)
    out_specs = (PartitionSpec("core"),) * n_outs
    donate = tuple(range(n_params, n_params + n_outs))
    sharded = jax.jit(
        shard_map(
            _body, mesh=mesh, in_specs=in_specs, out_specs=out_specs,
            check_rep=False,
        ),
        donate_argnums=donate,
        keep_unused=True,
    )
    ctx = {
        "fn": sharded,
        "in_names": in_names,
        "out_names": out_names,
        "zero_outs": zero_outs,
        "mesh": mesh,
    }
    _CACHE["exec"] = ctx
    return ctx


def _concat_inputs(in_maps, in_names):
    return [
        np.concatenate([in_maps[c][nm] for c in range(NCORES)], axis=0)
        for nm in in_names
    ]


def _run(ctx, concat_in):
    zeros = [
        np.zeros((NCORES * z.shape[0], *z.shape[1:]), z.dtype)
        for z in ctx["zero_outs"]
    ]
    return ctx["fn"](*concat_in, *zeros)


def kernel(x, W1, W3, W2, gate_w, A1, B1, A3, B3, A2, B2):
    ctx = _ensure_compiled()
    in_maps = _prep_inputs(x, W1, W3, W2, gate_w, A1, B1, A3, B3, A2, B2)
    concat_in = _concat_inputs(in_maps, ctx["in_names"])
    out_arrs = _run(ctx, concat_in)
    ntok = N // NCORES
    res = np.asarray(out_arrs[ctx["out_names"].index("out_t")])
    res = res.reshape(NCORES, D, ntok)
    out = np.empty((N, D), np.float32)
    for c in range(NCORES):
        out[c * ntok:(c + 1) * ntok, :] = res[c].T
    return out


def time_device(inputs, iters=3):
    """Upload inputs once, then wall-time jitted executions (per call)."""
    import time as _time

    import jax

    ctx = _ensure_compiled()
    in_maps = _prep_inputs(**inputs)
    concat_in = _concat_inputs(in_maps, ctx["in_names"])
    dev_in = jax.device_put(concat_in)  # default sharding: replicated? no —
    # put with the same sharding jit would choose: shard along axis 0
    from jax.sharding import NamedSharding, PartitionSpec

    sh = NamedSharding(ctx["mesh"], PartitionSpec("core"))
    dev_in = [jax.device_put(a, sh) for a in concat_in]
    # warmup (also compiles)
    jax.block_until_ready(_run(ctx, dev_in))
    times = []
    for _ in range(iters):
        t0 = _time.perf_counter()
        jax.block_until_ready(_run(ctx, dev_in))
        times.append(_time.perf_counter() - t0)
    return min(times)


# revision 39
# speedup vs baseline: 6.7226x; 6.7226x over previous
"""MixFFN MoE-routing kernel for Trainium2 (8 NeuronCores, token-parallel).

Math (per token block):
    logits = x @ gate_w.T ; probs = softmax(logits); top2 -> ew [N, E] (dense, rows sum to 1)
    CW1 = x @ W1.T ; CW3 = x @ W3.T
    per expert e:
        w1_e = CW1 + (x @ A1e.T) @ B1e.T
        w3_e = CW3 + (x @ A3e.T) @ B3e.T
        h_e  = silu(w1_e) * w3_e
    out = (sum_e ew_e * h_e) @ W2.T + sum_e ((ew_e * h_e) @ A2e.T) @ B2e.T

Key restructuring vs the reference scan: row-scaling by ew commutes with the
right matmuls, so the big W2 GEMM runs once on H = sum_e ew_e*h_e instead of
once per expert.  The u-projection (A2 contraction) is computed from the
unscaled h_e and column-scaled by ew afterwards (column scaling commutes with
contraction over dff).

Sharding: token-parallel.  Each of the 8 cores gets N/8 = 512 tokens and a
replicated copy of all weights; outputs are disjoint row blocks (no
collectives).  All layout transposes / dtype casts are done host-side.

On-chip layout: feature-on-partition ("transposed"), activations [feat, tok].
"""

import numpy as np

# problem dims (hardcoded per harness contract)
N, D, DFF, E, KTOP, R = 4096, 2048, 8192, 8, 2, 16
NCORES = 8
P = 128

_CACHE = {}


def build_bass(D_=D, DFF_=DFF, E_=E, R_=R, NTOK=N // NCORES):
    """Build the per-core Bass program (same SPMD program on every core)."""
    import concourse.bass as bass
    import concourse.mybir as mybir
    from concourse import bacc
    from concourse.tile import TileContext
    from concourse.masks import make_identity

    dt = mybir.dt
    op = mybir.AluOpType
    AF = mybir.ActivationFunctionType

    KD = D_ // P      # contraction tiles over D
    KF = DFF_ // P    # dff tiles
    MD = D_ // P      # output d tiles
    TT = NTOK // P    # token tiles
    ER = E_ * R_      # stacked expert-rank dim (=128 at full size)

    nc = bacc.Bacc("TRN2", target_bir_lowering=False, debug=False)

    # ---- DRAM I/O ----
    x_bf = nc.dram_tensor("x_bf", [D_, NTOK], dt.bfloat16, kind="ExternalInput")
    x_f = nc.dram_tensor("x_f", [D_, NTOK], dt.float32, kind="ExternalInput")
    gate = nc.dram_tensor("gate", [D_, E_], dt.float32, kind="ExternalInput")
    w1t = nc.dram_tensor("w1t", [D_, DFF_], dt.bfloat16, kind="ExternalInput")
    w3t = nc.dram_tensor("w3t", [D_, DFF_], dt.bfloat16, kind="ExternalInput")
    w2t = nc.dram_tensor("w2t", [DFF_, D_], dt.bfloat16, kind="ExternalInput")
    # A1/A3 packed even/odd with 32-aligned expert slots:
    # a1p[:, par, 32*j : 32*j+16] = A1[2*j+par].T  (zeros elsewhere)
    a1p = nc.dram_tensor("a1p", [D_, 2, P], dt.bfloat16, kind="ExternalInput")
    a3p = nc.dram_tensor("a3p", [D_, 2, P], dt.bfloat16, kind="ExternalInput")
    # B1/B3 packed even/odd with 32-aligned expert slots (rows 32j..32j+16 of
    # plane par hold B[2j+par].T), matching the T-projection psum layout so
    # pairs of delta-matmuls can row-pack via tile_position.
    b1s = nc.dram_tensor("b1s", [P, 2, DFF_], dt.bfloat16, kind="ExternalInput")
    b3s = nc.dram_tensor("b3s", [P, 2, DFF_], dt.bfloat16, kind="ExternalInput")
    a2s = nc.dram_tensor("a2s", [DFF_, ER], dt.bfloat16, kind="ExternalInput")
    b2s = nc.dram_tensor("b2s", [R_, E_, D_], dt.bfloat16, kind="ExternalInput")
    out_t = nc.dram_tensor("out_t", [D_, NTOK], dt.float32, kind="ExternalOutput")

    with TileContext(nc) as tc:
        with (
            tc.tile_pool(name="persist", bufs=1) as persist,
            tc.tile_pool(name="psum_cw", bufs=1, space="PSUM") as ppool_cw,
            tc.tile_pool(name="psum_d", bufs=2, space="PSUM") as ppool_d,
            tc.tile_pool(name="psum_u", bufs=1, space="PSUM") as ppool_u,
            tc.tile_pool(name="dram", bufs=1, space="DRAM") as dpool,
        ):
            # ---------- persistent tiles (created upfront so the persist
            # pool's footprint is settled before scoped pools stack above) ----
            xbf = persist.tile([P, KD, NTOK], dt.bfloat16)
            nc.sync.dma_start(out=xbf, in_=x_bf.rearrange("(k p) t -> p k t", p=P))
            ident_f = persist.tile([P, P], dt.float32)
            make_identity(nc, ident_f)
            # H accumulator (bf16) for the whole dff range
            h_big = persist.tile([P, KF, NTOK], dt.bfloat16)
            ewT_sb = persist.tile([E_, NTOK], dt.bfloat16)
            ew_b = []
            for e in range(E_):
                ewb_t = persist.tile([P, NTOK], dt.bfloat16, tag=f"ewb{e}")
                ew_b.append(ewb_t)
            t1p, t3p = [None, None], [None, None]
            for par in range(2):
                t1_t = persist.tile([P, NTOK], dt.bfloat16, tag=f"t1_{par}")
                t1p[par] = t1_t
                t3_t = persist.tile([P, NTOK], dt.bfloat16, tag=f"t3_{par}")
                t3p[par] = t3_t
            uw = []
            for e in range(E_):
                uw_t = persist.tile([R_, NTOK], dt.bfloat16, tag=f"uw{e}")
                uw.append(uw_t)

            # ---------- phase 0: routing + lora-down projections ----------
            ew_td = dpool.tile([E_, NTOK], dt.bfloat16)
            p0_cm = tc.tile_pool(name="p0", bufs=3)
            p0 = p0_cm.__enter__()
            xf = p0.tile([P, KD, NTOK], dt.float32, bufs=1)
            nc.sync.dma_start(out=xf, in_=x_f.rearrange("(k p) t -> p k t", p=P))
            gsb = p0.tile([P, KD, E_], dt.float32, bufs=1)
            nc.sync.dma_start(out=gsb, in_=gate.rearrange("(k p) e -> p k e", p=P))
            a1sb = p0.tile([P, KD, 2, P], dt.bfloat16, bufs=1)
            nc.sync.dma_start(
                out=a1sb, in_=a1p.rearrange("(k p) g c -> p k g c", p=P)
            )
            a3sb = p0.tile([P, KD, 2, P], dt.bfloat16, bufs=1)
            nc.sync.dma_start(
                out=a3sb, in_=a3p.rearrange("(k p) g c -> p k g c", p=P)
            )
            ew_pool = p0
            for tt in range(TT):
                lg = ppool_d.tile([P, E_], dt.float32, tag="d1")
                for k in range(KD):
                    nc.tensor.matmul(
                        lg,
                        lhsT=xf[:, k, tt * P:(tt + 1) * P],
                        rhs=gsb[:, k, :],
                        start=(k == 0),
                        stop=(k == KD - 1),
                    )
                l_sb = ew_pool.tile([P, E_], dt.float32, tag="lsb")
                nc.vector.tensor_copy(l_sb, lg)
                m1 = ew_pool.tile([P, 1], dt.float32, tag="m1")
                nc.vector.reduce_max(m1, l_sb, axis=mybir.AxisListType.X)
                nm1 = ew_pool.tile([P, 1], dt.float32, tag="nm1")
                nc.vector.tensor_scalar_mul(nm1, m1, -1.0)
                mask1 = ew_pool.tile([P, E_], dt.float32, tag="mask1")
                nc.vector.tensor_scalar(
                    mask1, l_sb, scalar1=m1, scalar2=None, op0=op.is_equal
                )
                l2 = ew_pool.tile([P, E_], dt.float32, tag="l2")
                # l2 = mask1 * (-1e30) + l
                nc.vector.scalar_tensor_tensor(
                    l2, in0=mask1, scalar=-1e30, in1=l_sb, op0=op.mult, op1=op.add
                )
                m2 = ew_pool.tile([P, 1], dt.float32, tag="m2")
                nc.vector.reduce_max(m2, l2, axis=mybir.AxisListType.X)
                mask2 = ew_pool.tile([P, E_], dt.float32, tag="mask2")
                nc.vector.tensor_scalar(
                    mask2, l2, scalar1=m2, scalar2=None, op0=op.is_equal
                )
                mask = ew_pool.tile([P, E_], dt.float32, tag="mask")
                nc.vector.tensor_tensor(mask, mask1, mask2, op=op.add)
                pexp = ew_pool.tile([P, E_], dt.float32, tag="pexp")
                nc.scalar.activation(pexp, l_sb, AF.Exp, bias=nm1, scale=1.0)
                pm = ew_pool.tile([P, E_], dt.float32, tag="pm")
                nc.vector.tensor_tensor(pm, pexp, mask, op=op.mult)
                den = ew_pool.tile([P, 1], dt.float32, tag="den")
                nc.vector.reduce_sum(den, pm, axis=mybir.AxisListType.X)
                rec = ew_pool.tile([P, 1], dt.float32, tag="rec")
                nc.vector.reciprocal(rec, den)
                ewt = ew_pool.tile([P, E_], dt.float32, tag="ewt")
                nc.vector.tensor_scalar_mul(ewt, pm, rec)
                # transpose [P, E] -> [E, P] and collect into ewT
                ewtp = ppool_d.tile([E_, P], dt.float32, tag="d3")
                nc.tensor.transpose(ewtp, ewt, ident_f)
                nc.scalar.copy(ewT_sb[:, tt * P:(tt + 1) * P], ewtp)

            nc.sync.dma_start(out=ew_td, in_=ewT_sb)
            # broadcast ew rows across partitions: EW_b[e] [P, NTOK]
            for e in range(E_):
                src = bass.AP(
                    tensor=ew_td.tensor,
                    offset=ew_td.offset + e * NTOK,
                    ap=[[0, P], [1, NTOK]],
                )
                nc.sync.dma_start(out=ew_b[e], in_=src)

            # ---------- T1/T3 = stacked per-expert lora-down projections ----------
            # expert 2*j+par sits at rows 32*j..32*j+16 of the `par` chain
            for asb, tlist in ((a1sb, t1p), (a3sb, t3p)):
                for par in range(2):
                    tp = ppool_d.tile([P, NTOK], dt.float32, tag="d1")
                    for k in range(KD):
                        nc.tensor.matmul(
                            tp,
                            lhsT=asb[:, k, par, :],
                            rhs=xbf[:, k, :],
                            start=(k == 0),
                            stop=(k == KD - 1),
                        )
                    nc.scalar.copy(tlist[par], tp)

            p0_cm.__exit__(None, None, None)
            stream_cm = tc.tile_pool(name="stream", bufs=2)
            stream = stream_cm.__enter__()

            # ---------- U accumulators (per-expert lora-up of h, unscaled) ----------
            u_ps_a = ppool_u.tile([P, NTOK], dt.float32, tag="ua")
            u_ps_b = ppool_u.tile([P, NTOK], dt.float32, tag="ub")
            u_ps = [u_ps_a, u_ps_b]

            # ---------- main dff loop ----------
            ttc = 0  # round-robin counter for p/q engine assignment
            for kt in range(KF):
                # CW1/CW3 for this dff tile
                w1sl = stream.tile([P, KD, P], dt.bfloat16, tag="w1sl")
                nc.sync.dma_start(
                    out=w1sl,
                    in_=w1t.rearrange("(k p) f -> p k f", p=P)[
                        :, :, kt * P:(kt + 1) * P
                    ],
                )
                w3sl = stream.tile([P, KD, P], dt.bfloat16, tag="w3sl")
                nc.sync.dma_start(
                    out=w3sl,
                    in_=w3t.rearrange("(k p) f -> p k f", p=P)[
                        :, :, kt * P:(kt + 1) * P
                    ],
                )
                cw1p = ppool_cw.tile([P, NTOK], dt.float32, tag="cw1")
                cw3p = ppool_cw.tile([P, NTOK], dt.float32, tag="cw3")
                for k in range(KD):
                    nc.tensor.matmul(
                        cw1p, lhsT=w1sl[:, k, :], rhs=xbf[:, k, :],
                        start=(k == 0), stop=(k == KD - 1),
                    )
                for k in range(KD):
                    nc.tensor.matmul(
                        cw3p, lhsT=w3sl[:, k, :], rhs=xbf[:, k, :],
                        start=(k == 0), stop=(k == KD - 1),
                    )
                cw1 = stream.tile([P, NTOK], dt.bfloat16, tag="cw1s")
                nc.scalar.copy(cw1, cw1p)
                cw3 = stream.tile([P, NTOK], dt.bfloat16, tag="cw3s")
                nc.scalar.copy(cw3, cw3p)

                # per-kt lora weights (even/odd packed, rows 32j hold B[2j+par].T)
                b1kt = stream.tile([P, 2, P], dt.bfloat16, tag="b1kt")
                nc.sync.dma_start(out=b1kt, in_=b1s[:, :, kt * P:(kt + 1) * P])
                b3kt = stream.tile([P, 2, P], dt.bfloat16, tag="b3kt")
                nc.sync.dma_start(out=b3kt, in_=b3s[:, :, kt * P:(kt + 1) * P])
                a2kt = stream.tile([P, ER], dt.bfloat16, tag="a2kt")
                nc.sync.dma_start(out=a2kt, in_=a2s[kt * P:(kt + 1) * P, :])

                hslice = h_big[:, kt, :]
                # pairs share a parity and differ in 32-row group, so the two
                # K=16 delta matmuls of a pair row-pack on the PE
                for e0, e1 in ((0, 2), (1, 3), (4, 6), (5, 7)):
                    dd = {}
                    for e in (e0, e1):
                        par, j = e % 2, e // 2
                        r0 = 32 * j
                        d1p = ppool_d.tile([P, NTOK], dt.float32, tag="d1")
                        nc.tensor.matmul(
                            d1p, lhsT=b1kt[r0:r0 + R_, par, :],
                            rhs=t1p[par][r0:r0 + R_, :],
                            start=True, stop=True, tile_position=(r0, 0),
                        )
                        d3p = ppool_d.tile([P, NTOK], dt.float32, tag="d3")
                        nc.tensor.matmul(
                            d3p, lhsT=b3kt[r0:r0 + R_, par, :],
                            rhs=t3p[par][r0:r0 + R_, :],
                            start=True, stop=True, tile_position=(r0, 0),
                        )
                        dd[e] = (d1p, d3p)
                    for e in (e0, e1):
                        d1p, d3p = dd[e]
                        j = e // 2
                        w1e = stream.tile([P, NTOK], dt.bfloat16, tag="w1e", bufs=3)
                        nc.vector.tensor_tensor(w1e, cw1, d1p, op=op.add)
                        s_e = stream.tile([P, NTOK], dt.bfloat16, tag="s_e", bufs=3)
                        nc.scalar.activation(s_e, w1e, AF.Silu)
                        w3e = stream.tile([P, NTOK], dt.bfloat16, tag="w3e", bufs=3)
                        nc.vector.tensor_tensor(w3e, cw3, d3p, op=op.add)
                        p_e = stream.tile([P, NTOK], dt.bfloat16, tag="p_e", bufs=3)
                        eng = nc.vector if ttc % 4 == 0 else nc.gpsimd
                        ttc += 1
                        eng.tensor_tensor(p_e, s_e, w3e, op=op.mult)
                        # q = ew * p ; first expert writes H directly, rest add
                        if e == 0:
                            eng = nc.gpsimd
                            eng.tensor_tensor(hslice, p_e, ew_b[e], op=op.mult)
                        else:
                            q_e = stream.tile(
                                [P, NTOK], dt.bfloat16, tag="q_e", bufs=3
                            )
                            eng = nc.vector if ttc % 4 == 0 else nc.gpsimd
                            ttc += 1
                            eng.tensor_tensor(q_e, p_e, ew_b[e], op=op.mult)
                            nc.vector.tensor_tensor(hslice, hslice, q_e, op=op.add)
                        # U[e] += A2e.T-contraction of (unscaled) p_e
                        nc.tensor.matmul(
                            u_ps[e % 2][32 * j:32 * j + R_, :],
                            lhsT=a2kt[:, e * R_:(e + 1) * R_],
                            rhs=p_e,
                            start=(kt == 0),
                            stop=(kt == KF - 1),
                            tile_position=(0, 32 * j),
                        )

            # ---------- Uw: apply ew column scaling to U ----------
            for e in range(E_):
                j = e // 2
                nc.vector.tensor_tensor(
                    uw[e], u_ps[e % 2][32 * j:32 * j + R_, :], ew_b[e][0:R_, :],
                    op=op.mult,
                )

            # ---------- output GEMM: out = W2 @ H + sum_e B2e @ Uw_e ----------
            KH = KF // 2
            w2r = w2t.rearrange("(k p) n -> p k n", p=P)
            for m in range(MD):
                outp = ppool_d.tile([P, NTOK], dt.float32, tag="d1")
                for h in range(2):
                    w2m = stream.tile([P, KH, P], dt.bfloat16, tag="w2m")
                    nc.sync.dma_start(
                        out=w2m,
                        in_=w2r[:, h * KH:(h + 1) * KH, m * P:(m + 1) * P],
                    )
                    for kk in range(KH):
                        kt = h * KH + kk
                        nc.tensor.matmul(
                            outp, lhsT=w2m[:, kk, :], rhs=h_big[:, kt, :],
                            start=(kt == 0), stop=False,
                        )
                b2m = stream.tile([R_, E_, P], dt.bfloat16, tag="b2m")
                nc.sync.dma_start(out=b2m, in_=b2s[:, :, m * P:(m + 1) * P])
                for e in range(E_):
                    nc.tensor.matmul(
                        outp, lhsT=b2m[:, e, :], rhs=uw[e],
                        start=False, stop=(e == E_ - 1),
                    )
                osb = stream.tile([P, NTOK], dt.float32, tag="osb")
                nc.scalar.copy(osb, outp)
                nc.sync.dma_start(out=out_t[m * P:(m + 1) * P, :], in_=osb)

            stream_cm.__exit__(None, None, None)

    nc.compile()
    return nc


def _pack_a_evenodd(A):
    """A [E, R, D] -> [D, 2, 128] with A[2j+par].T at [:, par, 32j:32j+16]."""
    E_, R_, D_ = A.shape
    out = np.zeros((D_, 2, 128), A.dtype)
    for e in range(E_):
        par, j = e % 2, e // 2
        out[:, par, 32 * j:32 * j + R_] = A[e].T
    return np.ascontiguousarray(out)


def _pack_b_evenodd(B):
    """B [E, F, R] -> [128, 2, F] with B[2j+par].T at [32j:32j+16, par, :]."""
    E_, F_, R_ = B.shape
    out = np.zeros((128, 2, F_), B.dtype)
    for e in range(E_):
        par, j = e % 2, e // 2
        out[32 * j:32 * j + R_, par, :] = B[e].T
    return np.ascontiguousarray(out)


def _prep_inputs(x, W1, W3, W2, gate_w, A1, B1, A3, B3, A2, B2):
    """Host-side packing: transposes + casts, shared across cores."""
    import ml_dtypes

    bf16 = ml_dtypes.bfloat16
    f32 = np.float32

    xT = np.ascontiguousarray(np.asarray(x, f32).T)            # [D, N]
    shared = {
        "gate": np.ascontiguousarray(np.asarray(gate_w, f32).T),   # [D, E]
        "w1t": np.ascontiguousarray(np.asarray(W1, f32).T.astype(bf16)),
        "w3t": np.ascontiguousarray(np.asarray(W3, f32).T.astype(bf16)),
        "w2t": np.ascontiguousarray(np.asarray(W2, f32).T.astype(bf16)),
        "a1p": _pack_a_evenodd(np.asarray(A1, f32)).astype(bf16),
        "a3p": _pack_a_evenodd(np.asarray(A3, f32)).astype(bf16),
        "b1s": _pack_b_evenodd(np.asarray(B1, f32)).astype(bf16),
        "b3s": _pack_b_evenodd(np.asarray(B3, f32)).astype(bf16),
        "a2s": np.ascontiguousarray(
            np.asarray(A2, f32).transpose(2, 0, 1).reshape(A2.shape[2], -1)
        ).astype(bf16),
        "b2s": np.ascontiguousarray(np.asarray(B2, f32).transpose(2, 0, 1)).astype(bf16),
    }
    ntok = xT.shape[1] // NCORES
    in_maps = []
    for c in range(NCORES):
        sl = np.ascontiguousarray(xT[:, c * ntok:(c + 1) * ntok])
        m = dict(shared)
        m["x_f"] = sl
        m["x_bf"] = sl.astype(bf16)
        in_maps.append(m)
    return in_maps


def _ensure_compiled():
    """Build the Bass program and a jitted 8-core shard_map executor.

    Mirrors concourse.bass2jax.run_bass_via_pjrt, but caches the jitted
    callable and keeps real inputs un-donated so device buffers can be
    reused across calls (for timing)."""
    if "exec" in _CACHE:
        return _CACHE["exec"]

    import jax
    import concourse.mybir as mybir
    from concourse import bass2jax
    from jax.experimental.shard_map import shard_map
    from jax.sharding import Mesh, PartitionSpec

    nc = build_bass()
    bass2jax.install_neuronx_cc_hook()

    partition_name = (
        nc.partition_id_tensor.name if nc.partition_id_tensor else None
    )
    in_names, out_names, out_avals, zero_outs = [], [], [], []
    for alloc in nc.m.functions[0].allocations:
        if not isinstance(alloc, mybir.MemoryLocationSet):
            continue
        name = alloc.memorylocations[0].name
        if alloc.kind == "ExternalInput":
            if name != partition_name:
                in_names.append(name)
        elif alloc.kind == "ExternalOutput":
            np_dtype = mybir.dt.np(alloc.dtype)
            out_names.append(name)
            out_avals.append(
                jax.core.ShapedArray(tuple(alloc.tensor_shape), np_dtype)
            )
            zero_outs.append(np.zeros(tuple(alloc.tensor_shape), np_dtype))

    n_params = len(in_names)
    n_outs = len(out_names)
    all_names = in_names + out_names
    if partition_name is not None:
        all_names = all_names + [partition_name]

    def _body(*args):
        operands = list(args)
        if partition_name is not None:
            operands.append(bass2jax.partition_id_tensor())
        outs = bass2jax._bass_exec_p.bind(
            *operands,
            out_avals=tuple(out_avals),
            in_names=tuple(all_names),
            out_names=tuple(out_names),
            lowering_input_output_aliases=(),
            sim_require_finite=True,
            sim_require_nnan=True,
            nc=nc,
        )
        return tuple(outs)

    devices = jax.devices()[:NCORES]
    mesh = Mesh(np.asarray(devices), ("core",))
    in_specs = (PartitionSpec("core"),) * (n_params + n_outs)
    out_specs = (PartitionSpec("core"),) * n_outs
    donate = tuple(range(n_params, n_params + n_outs))
    sharded = jax.jit(
        shard_map(
            _body, mesh=mesh, in_specs=in_specs, out_specs=out_specs,
            check_rep=False,
        ),
        donate_argnums=donate,
        keep_unused=True,
    )
    ctx = {
        "fn": sharded,
        "body": _body,
        "n_operands": n_params + n_outs,
        "in_names": in_names,
        "out_names": out_names,
        "zero_outs": zero_outs,
        "mesh": mesh,
    }
    _CACHE["exec"] = ctx
    return ctx


def _concat_inputs(in_maps, in_names):
    return [
        np.concatenate([in_maps[c][nm] for c in range(NCORES)], axis=0)
        for nm in in_names
    ]


def _run(ctx, concat_in):
    zeros = [
        np.zeros((NCORES * z.shape[0], *z.shape[1:]), z.dtype)
        for z in ctx["zero_outs"]
    ]
    return ctx["fn"](*concat_in, *zeros)


def kernel(x, W1, W3, W2, gate_w, A1, B1, A3, B3, A2, B2):
    ctx = _ensure_compiled()
    in_maps = _prep_inputs(x, W1, W3, W2, gate_w, A1, B1, A3, B3, A2, B2)
    concat_in = _concat_inputs(in_maps, ctx["in_names"])
    out_arrs = _run(ctx, concat_in)
    ntok = N // NCORES
    res = np.asarray(out_arrs[ctx["out_names"].index("out_t")])
    res = res.reshape(NCORES, D, ntok)
    out = np.empty((N, D), np.float32)
    for c in range(NCORES):
        out[c * ntok:(c + 1) * ntok, :] = res[c].T
    return out


def time_device(inputs, iters=3):
    """Upload all operands once (no donation), then wall-time jitted runs."""
    import time as _time

    import jax
    from jax.experimental.shard_map import shard_map
    from jax.sharding import NamedSharding, PartitionSpec, Mesh

    ctx = _ensure_compiled()
    if "fn_nodonate" not in ctx:
        ctx["fn_nodonate"] = jax.jit(
            shard_map(
                ctx["body"], mesh=ctx["mesh"],
                in_specs=(PartitionSpec("core"),) * ctx["n_operands"],
                out_specs=(PartitionSpec("core"),) * len(ctx["out_names"]),
                check_rep=False,
            ),
            keep_unused=True,
        )
    fn = ctx["fn_nodonate"]
    in_maps = _prep_inputs(**inputs)
    concat_in = _concat_inputs(in_maps, ctx["in_names"])
    zeros = [
        np.zeros((NCORES * z.shape[0], *z.shape[1:]), z.dtype)
        for z in ctx["zero_outs"]
    ]
    sh = NamedSharding(ctx["mesh"], PartitionSpec("core"))
    dev = [jax.device_put(a, sh) for a in (concat_in + zeros)]
    jax.block_until_ready(fn(*dev))  # warmup/compile
    times = []
    for _ in range(iters):
        t0 = _time.perf_counter()
        jax.block_until_ready(fn(*dev))
        times.append(_time.perf_counter() - t0)
    return min(times)
